# revision 31
# baseline (speedup 1.0000x reference)
"""Trainium2 Bass kernel for the DActor dense MLP.

Network (per row of `state`):
    h1 = relu(state @ W1 + b1)        # 512 -> 500
    h2 = relu(h1 @ W2 + b2)           # 500 -> 300
    h3 = relu(h2 @ W3 + b3)           # 300 -> 100
    v  = h3 @ W4 + b4                 # 100 -> 64
    t  = tanh(v[:, :63]); s = sigmoid(v[:, 63:])
    possum = sum(relu(t)); denom = possum == 0 ? 1 : possum
    out = concat(where(t > 0, t / denom, t), s)

Strategy: pure data parallel over 8 NeuronCores (8192 rows each).
Activations are feature-major ([feat, batch]); weights are the stationary
operand, activations the 512-wide moving operand. All matmul operands are
bf16 (fp32 PSUM accumulation) — same 1 col/cycle PE rate as fp32r but half
the DMA/LDWEIGHTS traffic.

Biases ride inside the matmuls via a ones-row that propagates through the
net: b1 is applied by the ACT engine whose padded bias vector also plants
h1[500] = relu(0 + 1) = 1; W2/W3 are padded with a bias row (row 500/300)
plus a 1.0 diagonal element that regenerates the ones-row in h2/h3. L2/L3
activations are then pure relu and run on the DVE.

L4 is fused with the batch-transpose: per 128-row block, the matmul uses
the feature-major h3 block as the *stationary* operand and W4 (with b4 as
its ones-row) as the 64-column moving operand, writing v4 batch-major
straight into PSUM — no identity transpose, no separate bias pass. The PST
epilogue computes out = t + relu(t) * (1/denom - 1), which equals
where(t>0, t/denom, t) without predicated copies.

The output DRAM tensor is partition-major [128, 16, 8, 64] so each store
is 2KB-contiguous per partition; the host inverts the permutation.
"""

import os

import ml_dtypes
import numpy as np

import concourse.bass as bass
import concourse.tile as tile
from concourse import bacc, mybir
from concourse.bass_utils import run_bass_kernel_spmd

N_CORES = 8
BATCH = 65536
B = BATCH // N_CORES  # 8192 rows per core
D_IN, H1, H2, H3, D_OUT = 512, 500, 300, 100, 64
NCHUNK = 512  # moving-operand width (= 1 PSUM bank of fp32)
N_CHUNKS = B // NCHUNK  # 16
BLOCKS_PER_BM = 8  # 128-row blocks per batch-major output tile
N_BM = B // (128 * BLOCKS_PER_BM)  # 8 output tiles... 8192/1024 = 8? no: 8

F32 = mybir.dt.float32
BF16 = mybir.dt.bfloat16

K1, K2, K3 = 4, 4, 3  # k-tiles per layer (501->4x128, 301->3x128 incl bias row)
M1, M2 = 4, 3  # m-tiles for L1 (512 cols) / L2 (384 cols)


def _emit(tc: tile.TileContext, aps: dict):
    nc = tc.nc
    xT = aps["xT"]
    out = aps["out"]  # [128, N_CHUNKS, ...] partition-major, see _build

    consts = tc.alloc_tile_pool(name="consts", bufs=1)
    acts = tc.alloc_tile_pool(name="acts", bufs=3)
    outs = tc.alloc_tile_pool(name="outs", bufs=3)
    scratch = tc.alloc_tile_pool(name="scratch", bufs=2)
    psum_mm = tc.alloc_tile_pool(name="psum_mm", bufs=7, space="PSUM")
    psum_bm = tc.alloc_tile_pool(name="psum_bm", bufs=1, space="PSUM")

    # ---- persistent constants -------------------------------------------
    # Weights arrive host-padded (see kernel()): W1 [512,512], W2 [512,384]
    # with row 500 = [b2, 1.0@300], W3 [384,128] with row 300 = [b3, 1.0@100],
    # W4 [128,64] with row 100 = b4. Padded rows/cols are zero so every
    # matmul runs full-K with bit-identical results.
    xT_v = xT.rearrange("(k p) b -> p k b", p=128)  # [128, 4, B]

    # Each dma_start owns one DMA ring (~24GB/s) and costs ~750ns of
    # descriptor-generation on the issuing sequencer, so the first-chunk
    # data (x0 + w1) is split per k-tile across 8 rings, k0 first. w2..w4/b1
    # ride the gpsimd queue and are issued after chunk 0's layer-1 emission
    # (not needed until ~+8us).
    x0_sb = acts.tile([128, K1, NCHUNK], BF16, tag="x", name="x0_sb")
    w1 = consts.tile([128, K1, 512], BF16)
    w1_v = aps["W1"].rearrange("(k p) m -> p k m", p=128)
    for ki in range(K1):
        nc.sync.dma_start(out=x0_sb[:, ki, :], in_=xT_v[:, ki, 0:NCHUNK])
        nc.sync.dma_start(out=w1[:, ki, :], in_=w1_v[:, ki, :])
    w2 = consts.tile([128, K2, 384], BF16)
    w3 = consts.tile([128, K3, 128], BF16)
    w4 = consts.tile([128, D_OUT], BF16)
    # b1 padded with b1[500] = 1.0: relu(psum_pad + 1) plants the ones-row
    # in h1 that carries the biases through L2/L3/L4.
    b1 = consts.tile([128, M1], F32)

    def load_late_consts():
        # split across rings: a single dma_start owns one ~24GB/s ring and
        # processes descriptors at ~42ns each, so the 512-descriptor w2/b1
        # transfers would otherwise land ~20us late
        b1_v = aps["b1"].rearrange("(m p) -> p m", p=128)
        for mi in range(M1):
            nc.gpsimd.dma_start(out=b1[:, mi:mi + 1], in_=b1_v[:, mi:mi + 1])
        w2_v = aps["W2"].rearrange("(k p) m -> p k m", p=128)
        for ki in range(K2):
            nc.gpsimd.dma_start(out=w2[:, ki, :], in_=w2_v[:, ki, :])
        w3_v = aps["W3"].rearrange("(k p) m -> p k m", p=128)
        for ki in range(K3):
            nc.gpsimd.dma_start(out=w3[:, ki, :], in_=w3_v[:, ki, :])
        nc.gpsimd.dma_start(out=w4, in_=aps["W4"])

    Relu = mybir.ActivationFunctionType.Relu

    # Warm the PE while the first DMAs land: the tensor engine's DVFS needs
    # ~3us of continuous execution to reach full clock, and these junk
    # matmuls (zero stationary/moving, result never read) have no input
    # dependencies, so the PE ramps during the DMA head instead of on the
    # first real chunks.
    wu = consts.tile([128, 64], BF16)
    nc.gpsimd.memset(wu, 0.0)
    wu_ps = psum_bm.tile([128, 8, D_OUT], F32, tag="bm", name="wu_ps")
    for i in range(90):
        nc.tensor.matmul(wu_ps[:64, i % 8, :], wu, wu, start=True, stop=True)

    def mm(ps, lhsT, rhs, start, stop):
        nc.tensor.matmul(ps, lhsT, rhs, start=start, stop=stop)

    pending_l4 = None  # (chunk, h3 tile) whose L4 matmuls are deferred
    bm_state = [None]  # current batch-major psum tile

    def emit_l4():
        # L4 for the previous chunk, emitted after the next chunk's L1
        # matmuls so the PE never waits on the DVE-produced h3. Stationary
        # operand is the feature-major h3 block; moving operand is W4
        # (64 cols), writing v4+b4 batch-major straight into PSUM.
        nonlocal pending_l4
        if pending_l4 is None:
            return
        pc, ph3 = pending_l4
        pending_l4 = None
        for bb in range(NCHUNK // 128):
            g = pc * (NCHUNK // 128) + bb  # global 128-row block index
            t = g % BLOCKS_PER_BM
            if t == 0:
                bm_state[0] = psum_bm.tile([128, BLOCKS_PER_BM, D_OUT], F32,
                                           tag="bm", name="bm")
            bm = bm_state[0]
            mm(bm[:, t, :], ph3[:, bb * 128:(bb + 1) * 128], w4,
               start=True, stop=True)
            # PST runs in two half-groups: the first half fires a chunk
            # earlier (hidden under compute), halving the serial tail after
            # the final matmul.
            if t == BLOCKS_PER_BM // 2 - 1:
                _pst(nc, scratch, outs, bm, 0, g // BLOCKS_PER_BM)
            elif t == BLOCKS_PER_BM - 1:
                j = g // BLOCKS_PER_BM
                o_sb = _pst(nc, scratch, outs, bm, 1, j)
                if j == B // (128 * BLOCKS_PER_BM) - 1:
                    # the final store is tail-critical: split it across 4
                    # rings (a single ring moves only ~24GB/s), with the
                    # ~750ns trigger generation split across two sequencers
                    for q in range(4):
                        eng = nc.sync if q % 2 == 0 else nc.scalar
                        eng.dma_start(out=out[32 * q:32 * (q + 1), j],
                                      in_=o_sb[32 * q:32 * (q + 1)])
                else:
                    nc.sync.dma_start(out=out[:, j], in_=o_sb)

    for c in range(N_CHUNKS):
        cs = slice(c * NCHUNK, (c + 1) * NCHUNK)

        if c == 0:
            x_sb = x0_sb
        else:
            # Rings process FIFO, so these queue behind the chunk-0 data
            # without delaying it.
            x_sb = acts.tile([128, K1, NCHUNK], BF16, tag="x")
            for ki in range(K1):
                nc.sync.dma_start(out=x_sb[:, ki, :], in_=xT_v[:, ki, cs])

        # Each (k, m) pass is emitted as two 256-col half-matmuls: a matmul
        # whose output spans a full 512-element PSUM bank throttles the PE
        # to ~2.0GHz (measured), while <=256-col outputs stream at the full
        # ~2.4GHz even in back-to-back accumulation chains.
        HALF = NCHUNK // 2

        # ---- layer 1: [512 -> 500(+ones row, pad 512)] ------------------
        h1 = acts.tile([128, K2, NCHUNK], BF16, tag="h1")
        for mi in range(M1):
            ps = psum_mm.tile([128, NCHUNK], F32, tag="ps")
            msl = slice(mi * 128, (mi + 1) * 128)
            for h in range(2):
                hs = slice(h * HALF, (h + 1) * HALF)
                for ki in range(K1):
                    mm(ps[:, hs], w1[:, ki, msl], x_sb[:, ki, hs],
                       start=(ki == 0), stop=(ki == K1 - 1))
            nc.scalar.activation(out=h1[:, mi, :], in_=ps, func=Relu,
                                 bias=b1[:, mi:mi + 1])
        if c == 0:
            load_late_consts()
        emit_l4()

        # ---- layer 2: [501 -> 300(+ones row, pad 384)] ------------------
        h2 = acts.tile([128, K3, NCHUNK], BF16, tag="h2")
        for mi in range(M2):
            ps = psum_mm.tile([128, NCHUNK], F32, tag="ps")
            msl = slice(mi * 128, (mi + 1) * 128)
            for h in range(2):
                hs = slice(h * HALF, (h + 1) * HALF)
                for ki in range(K2):
                    mm(ps[:, hs], w2[:, ki, msl], h1[:, ki, hs],
                       start=(ki == 0), stop=(ki == K2 - 1))
            nc.vector.tensor_scalar_max(h2[:, mi, :], ps, 0.0)

        # ---- layer 3: [301 -> 100(+ones row, pad 128)] ------------------
        h3 = acts.tile([128, NCHUNK], BF16, tag="h3")
        ps = psum_mm.tile([128, NCHUNK], F32, tag="ps")
        for h in range(2):
            hs = slice(h * HALF, (h + 1) * HALF)
            for ki in range(K3):
                mm(ps[:, hs], w3[:, ki, :], h2[:, ki, hs],
                   start=(ki == 0), stop=(ki == K3 - 1))
        nc.vector.tensor_scalar_max(h3, ps, 0.0)

        pending_l4 = (c, h3)

    emit_l4()

    for pool in (psum_bm, psum_mm, scratch, outs, acts, consts):
        pool.release()


_PST_STATE = {}


def _pst(nc, scratch, outs, bm, half, j):
    """PST epilogue on half of a batch-major [128, 8, 64] PSUM tile.

    `bm` holds v4+b4 batch-major. out = t + relu(t)*(1/denom - 1), which
    equals where(t>0, t/denom, t): for t<=0 relu(t)=0; when possum==0 the
    denom fix makes the correction factor 0. Returns the output SBUF tile
    (complete when half == 1).
    """
    G = BLOCKS_PER_BM // 2
    Tanh = mybir.ActivationFunctionType.Tanh
    Sigm = mybir.ActivationFunctionType.Sigmoid

    if half == 0:
        _PST_STATE[j] = outs.tile([128, 2 * G, D_OUT], F32, tag="o", name="o_sb")
    o_full = _PST_STATE.pop(j) if half == 1 else _PST_STATE[j]
    o_sb = o_full[:, half * G:(half + 1) * G, :]
    bmh = bm[:, half * G:(half + 1) * G, :]

    nc.scalar.activation(out=o_sb[:, :, 0:63], in_=bmh[:, :, 0:63], func=Tanh)
    nc.scalar.activation(out=o_sb[:, :, 63:64], in_=bmh[:, :, 63:64], func=Sigm)

    tv = o_sb[:, :, 0:63]  # tanh part [128, G, 63]
    # rl/corr must stay f32: out = t + rl*rm1 cancels to ~t/15, so any
    # rounding in rl or corr is amplified ~30x on the positive outputs.
    rl = scratch.tile([128, G, 63], F32, tag="rl")
    nc.vector.tensor_scalar_max(rl, tv, 0.0)
    possum = scratch.tile([128, G], F32, tag="possum")
    nc.vector.reduce_sum(out=possum, in_=rl, axis=mybir.AxisListType.X)
    denom = scratch.tile([128, G], F32, tag="denom")
    nc.vector.scalar_tensor_tensor(out=denom, in0=possum, scalar=0.0,
                                   in1=possum, op0=mybir.AluOpType.is_equal,
                                   op1=mybir.AluOpType.add)
    recip = scratch.tile([128, G], F32, tag="recip")
    nc.vector.reciprocal(recip, denom)
    rm1 = scratch.tile([128, G], F32, tag="rm1")
    nc.vector.tensor_scalar(out=rm1, in0=recip, scalar1=-1.0, scalar2=None,
                            op0=mybir.AluOpType.add)
    corr = scratch.tile([128, G, 63], F32, tag="corr")
    nc.vector.tensor_tensor(
        out=corr, in0=rl, in1=rm1.unsqueeze(2).broadcast_to([128, G, 63]),
        op=mybir.AluOpType.mult)
    nc.vector.tensor_tensor(out=o_sb[:, :, 0:63], in0=tv, in1=corr,
                            op=mybir.AluOpType.add)
    return o_full


_PROG_CACHE = {}


def _build():
    if "nc" in _PROG_CACHE:
        return _PROG_CACHE["nc"]
    nc = bacc.Bacc("TRN2", target_bir_lowering=False, debug=False,
                   enable_asserts=False)
    n_j = B // (128 * BLOCKS_PER_BM)  # 8 output tiles of 1024 rows
    aps = {
        "xT": nc.dram_tensor("xT", [D_IN, B], BF16, kind="ExternalInput").ap(),
        "W1": nc.dram_tensor("W1", [D_IN, 512], BF16, kind="ExternalInput").ap(),
        "b1": nc.dram_tensor("b1", [512], F32, kind="ExternalInput").ap(),
        "W2": nc.dram_tensor("W2", [512, 384], BF16, kind="ExternalInput").ap(),
        "W3": nc.dram_tensor("W3", [384, 128], BF16, kind="ExternalInput").ap(),
        "W4": nc.dram_tensor("W4", [128, D_OUT], BF16, kind="ExternalInput").ap(),
        # partition-major: out[p, j, t, f] = row 1024*j + 128*t + p
        "out": nc.dram_tensor("out", [128, n_j, BLOCKS_PER_BM, D_OUT], F32,
                              kind="ExternalOutput").ap(),
    }
    with tile.TileContext(nc) as tc:
        _emit(tc, aps)
    nc.compile()
    _PROG_CACHE["nc"] = nc
    return nc


def kernel(state, W1, b1, W2, b2, W3, b3, W4, b4, _trace=False):
    nc = _build()
    state = np.asarray(state, dtype=np.float32)

    bf16 = ml_dtypes.bfloat16
    f32 = np.float32

    W1p = np.zeros((512, 512), f32)
    W1p[:, :H1] = np.asarray(W1, f32)
    b1p = np.zeros((512,), f32)
    b1p[:H1] = np.asarray(b1, f32)
    b1p[H1] = 1.0  # plants the ones-row in h1
    W2p = np.zeros((512, 384), f32)
    W2p[:H1, :H2] = np.asarray(W2, f32)
    W2p[H1, :H2] = np.asarray(b2, f32)
    W2p[H1, H2] = 1.0  # regenerates the ones-row in h2
    W3p = np.zeros((384, 128), f32)
    W3p[:H2, :H3] = np.asarray(W3, f32)
    W3p[H2, :H3] = np.asarray(b3, f32)
    W3p[H2, H3] = 1.0  # regenerates the ones-row in h3
    W4p = np.zeros((128, D_OUT), f32)
    W4p[:H3] = np.asarray(W4, f32)
    W4p[H3] = np.asarray(b4, f32)

    weights = {
        "W1": W1p.astype(bf16), "b1": b1p,
        "W2": W2p.astype(bf16), "W3": W3p.astype(bf16), "W4": W4p.astype(bf16),
    }
    in_maps = []
    for i in range(N_CORES):
        shard = state[i * B:(i + 1) * B]
        in_maps.append({"xT": np.ascontiguousarray(shard.T).astype(bf16),
                        **weights})

    res = run_bass_kernel_spmd(nc, in_maps, core_ids=list(range(N_CORES)),
                               trace=_trace)
    # invert the partition-major output layout: [128, j, t, f] -> row
    # 1024*j + 128*t + p
    full = np.concatenate(
        [np.transpose(res.results[i]["out"], (1, 2, 0, 3)).reshape(B, D_OUT)
         for i in range(N_CORES)], axis=0)
    if _trace:
        kernel.last_results = res
    return full


# revision 32
# speedup vs baseline: 1.1745x; 1.1745x over previous
"""Trainium2 Bass kernel for the DActor dense MLP.

Network (per row of `state`):
    h1 = relu(state @ W1 + b1)        # 512 -> 500
    h2 = relu(h1 @ W2 + b2)           # 500 -> 300
    h3 = relu(h2 @ W3 + b3)           # 300 -> 100
    v  = h3 @ W4 + b4                 # 100 -> 64
    t  = tanh(v[:, :63]); s = sigmoid(v[:, 63:])
    possum = sum(relu(t)); denom = possum == 0 ? 1 : possum
    out = concat(where(t > 0, t / denom, t), s)

Strategy: pure data parallel over 8 NeuronCores (8192 rows each).
Activations are feature-major ([feat, batch]); weights are the stationary
operand, activations the 512-wide moving operand. All matmul operands are
bf16 (fp32 PSUM accumulation) — same 1 col/cycle PE rate as fp32r but half
the DMA/LDWEIGHTS traffic.

Biases ride inside the matmuls via a ones-row that propagates through the
net: b1 is applied by the ACT engine whose padded bias vector also plants
h1[500] = relu(0 + 1) = 1; W2/W3 are padded with a bias row (row 500/300)
plus a 1.0 diagonal element that regenerates the ones-row in h2/h3. L2/L3
activations are then pure relu and run on the DVE.

L4 is fused with the batch-transpose: per 128-row block, the matmul uses
the feature-major h3 block as the *stationary* operand and W4 (with b4 as
its ones-row) as the 64-column moving operand, writing v4 batch-major
straight into PSUM — no identity transpose, no separate bias pass. The PST
epilogue computes out = t + relu(t) * (1/denom - 1), which equals
where(t>0, t/denom, t) without predicated copies.

The output DRAM tensor is partition-major [128, 16, 8, 64] so each store
is 2KB-contiguous per partition; the host inverts the permutation.
"""

import os

import ml_dtypes
import numpy as np

import concourse.bass as bass
import concourse.tile as tile
from concourse import bacc, mybir
from concourse.bass_utils import run_bass_kernel_spmd

N_CORES = 8
BATCH = 65536
B = BATCH // N_CORES  # 8192 rows per core
D_IN, H1, H2, H3, D_OUT = 512, 500, 300, 100, 64
NCHUNK = 512  # moving-operand width (= 1 PSUM bank of fp32)
N_CHUNKS = B // NCHUNK  # 16
BLOCKS_PER_BM = 8  # 128-row blocks per batch-major output tile
N_BM = B // (128 * BLOCKS_PER_BM)  # 8 output tiles... 8192/1024 = 8? no: 8

F32 = mybir.dt.float32
BF16 = mybir.dt.bfloat16

K1, K2, K3 = 4, 4, 3  # k-tiles per layer (501->4x128, 301->3x128 incl bias row)
M1, M2 = 4, 3  # m-tiles for L1 (512 cols) / L2 (384 cols)


def _emit(tc: tile.TileContext, aps: dict):
    nc = tc.nc
    xT = aps["xT"]
    out = aps["out"]  # [128, N_CHUNKS, ...] partition-major, see _build

    consts = tc.alloc_tile_pool(name="consts", bufs=1)
    acts = tc.alloc_tile_pool(name="acts", bufs=3)
    outs = tc.alloc_tile_pool(name="outs", bufs=3)
    scratch = tc.alloc_tile_pool(name="scratch", bufs=2)
    psum_mm = tc.alloc_tile_pool(name="psum_mm", bufs=7, space="PSUM")
    psum_bm = tc.alloc_tile_pool(name="psum_bm", bufs=1, space="PSUM")

    # ---- persistent constants -------------------------------------------
    # Weights arrive host-padded (see kernel()): W1 [512,512], W2 [512,384]
    # with row 500 = [b2, 1.0@300], W3 [384,128] with row 300 = [b3, 1.0@100],
    # W4 [128,64] with row 100 = b4. Padded rows/cols are zero so every
    # matmul runs full-K with bit-identical results.
    xT_v = xT.rearrange("(k p) b -> p k b", p=128)  # [128, 4, B]

    # Each dma_start owns one DMA ring (~24GB/s) and costs ~750ns of
    # descriptor-generation on the issuing sequencer, so the first-chunk
    # data (x0 + w1) is split per k-tile across 8 rings, k0 first. w2..w4/b1
    # ride the gpsimd queue and are issued after chunk 0's layer-1 emission
    # (not needed until ~+8us).
    x0_sb = acts.tile([128, K1, NCHUNK], BF16, tag="x", name="x0_sb")
    w1 = consts.tile([128, K1, 512], BF16)
    w1_v = aps["W1"].rearrange("(k p) m -> p k m", p=128)
    for ki in range(K1):
        nc.sync.dma_start(out=x0_sb[:, ki, :], in_=xT_v[:, ki, 0:NCHUNK])
        nc.sync.dma_start(out=w1[:, ki, :], in_=w1_v[:, ki, :])
    w2 = consts.tile([128, K2, 384], BF16)
    w3 = consts.tile([128, K3, 128], BF16)
    w4 = consts.tile([128, D_OUT], BF16)
    # b1 padded with b1[500] = 1.0: relu(psum_pad + 1) plants the ones-row
    # in h1 that carries the biases through L2/L3/L4.
    b1 = consts.tile([128, M1], F32)

    def load_late_consts():
        # split across rings: a single dma_start owns one ~24GB/s ring and
        # processes descriptors at ~42ns each, so the 512-descriptor w2/b1
        # transfers would otherwise land ~20us late
        b1_v = aps["b1"].rearrange("(m p) -> p m", p=128)
        for mi in range(M1):
            nc.gpsimd.dma_start(out=b1[:, mi:mi + 1], in_=b1_v[:, mi:mi + 1])
        w2_v = aps["W2"].rearrange("(k p) m -> p k m", p=128)
        for ki in range(K2):
            nc.gpsimd.dma_start(out=w2[:, ki, :], in_=w2_v[:, ki, :])
        w3_v = aps["W3"].rearrange("(k p) m -> p k m", p=128)
        for ki in range(K3):
            nc.gpsimd.dma_start(out=w3[:, ki, :], in_=w3_v[:, ki, :])
        nc.gpsimd.dma_start(out=w4, in_=aps["W4"])

    Relu = mybir.ActivationFunctionType.Relu

    # Warm the PE while the first DMAs land: the tensor engine's DVFS needs
    # ~3us of continuous execution to reach full clock, and these junk
    # matmuls (zero stationary/moving, result never read) have no input
    # dependencies, so the PE ramps during the DMA head instead of on the
    # first real chunks.
    wu = consts.tile([128, 64], BF16)
    nc.gpsimd.memset(wu, 0.0)
    wu_ps = psum_bm.tile([128, 8, D_OUT], F32, tag="bm", name="wu_ps")
    for i in range(110):
        nc.tensor.matmul(wu_ps[:64, i % 8, :], wu, wu, start=True, stop=True)

    def mm(ps, lhsT, rhs, start, stop):
        nc.tensor.matmul(ps, lhsT, rhs, start=start, stop=stop)

    pending_l4 = None  # (chunk, h3 tile) whose L4 matmuls are deferred
    bm_state = [None]  # current batch-major psum tile

    def emit_l4():
        # L4 for the previous chunk, emitted after the next chunk's L1
        # matmuls so the PE never waits on the DVE-produced h3. Stationary
        # operand is the feature-major h3 block; moving operand is W4
        # (64 cols), writing v4+b4 batch-major straight into PSUM.
        nonlocal pending_l4
        if pending_l4 is None:
            return
        pc, ph3 = pending_l4
        pending_l4 = None
        for bb in range(NCHUNK // 128):
            g = pc * (NCHUNK // 128) + bb  # global 128-row block index
            t = g % BLOCKS_PER_BM
            if t == 0:
                bm_state[0] = psum_bm.tile([128, BLOCKS_PER_BM, D_OUT], F32,
                                           tag="bm", name="bm")
            bm = bm_state[0]
            mm(bm[:, t, :], ph3[:, bb * 128:(bb + 1) * 128], w4,
               start=True, stop=True)
            # PST runs in two half-groups: the first half fires a chunk
            # earlier (hidden under compute), halving the serial tail after
            # the final matmul.
            if t == BLOCKS_PER_BM // 2 - 1:
                _pst(nc, scratch, outs, bm, 0, g // BLOCKS_PER_BM)
            elif t == BLOCKS_PER_BM - 1:
                j = g // BLOCKS_PER_BM
                o_sb = _pst(nc, scratch, outs, bm, 1, j)
                if j == B // (128 * BLOCKS_PER_BM) - 1:
                    # the final store is tail-critical: split it across 4
                    # rings (a single ring moves only ~24GB/s), with the
                    # ~750ns trigger generation split across two sequencers
                    for q in range(4):
                        eng = nc.sync if q % 2 == 0 else nc.scalar
                        eng.dma_start(out=out[32 * q:32 * (q + 1), j],
                                      in_=o_sb[32 * q:32 * (q + 1)])
                else:
                    nc.sync.dma_start(out=out[:, j], in_=o_sb)

    for c in range(N_CHUNKS):
        cs = slice(c * NCHUNK, (c + 1) * NCHUNK)

        if c == 0:
            x_sb = x0_sb
        else:
            # Rings process FIFO, so these queue behind the chunk-0 data
            # without delaying it.
            x_sb = acts.tile([128, K1, NCHUNK], BF16, tag="x")
            for ki in range(K1):
                nc.sync.dma_start(out=x_sb[:, ki, :], in_=xT_v[:, ki, cs])

        # Each (k, m) pass is emitted as two 256-col half-matmuls: a matmul
        # whose output spans a full 512-element PSUM bank throttles the PE
        # to ~2.0GHz (measured), while <=256-col outputs stream at the full
        # ~2.4GHz even in back-to-back accumulation chains.
        HALF = NCHUNK // 2

        # ---- layer 1: [512 -> 500(+ones row, pad 512)] ------------------
        h1 = acts.tile([128, K2, NCHUNK], BF16, tag="h1")
        for mi in range(M1):
            ps = psum_mm.tile([128, NCHUNK], F32, tag="ps")
            msl = slice(mi * 128, (mi + 1) * 128)
            for h in range(2):
                hs = slice(h * HALF, (h + 1) * HALF)
                for ki in range(K1):
                    mm(ps[:, hs], w1[:, ki, msl], x_sb[:, ki, hs],
                       start=(ki == 0), stop=(ki == K1 - 1))
            nc.scalar.activation(out=h1[:, mi, :], in_=ps, func=Relu,
                                 bias=b1[:, mi:mi + 1])
        if c == 0:
            load_late_consts()
        emit_l4()

        # ---- layer 2: [501 -> 300(+ones row, pad 384)] ------------------
        h2 = acts.tile([128, K3, NCHUNK], BF16, tag="h2")
        for mi in range(M2):
            ps = psum_mm.tile([128, NCHUNK], F32, tag="ps")
            msl = slice(mi * 128, (mi + 1) * 128)
            for h in range(2):
                hs = slice(h * HALF, (h + 1) * HALF)
                for ki in range(K2):
                    mm(ps[:, hs], w2[:, ki, msl], h1[:, ki, hs],
                       start=(ki == 0), stop=(ki == K2 - 1))
            nc.vector.tensor_scalar_max(h2[:, mi, :], ps, 0.0)

        # ---- layer 3: [301 -> 100(+ones row, pad 128)] ------------------
        h3 = acts.tile([128, NCHUNK], BF16, tag="h3")
        ps = psum_mm.tile([128, NCHUNK], F32, tag="ps")
        for h in range(2):
            hs = slice(h * HALF, (h + 1) * HALF)
            for ki in range(K3):
                mm(ps[:, hs], w3[:, ki, :], h2[:, ki, hs],
                   start=(ki == 0), stop=(ki == K3 - 1))
        nc.vector.tensor_scalar_max(h3, ps, 0.0)

        pending_l4 = (c, h3)

    emit_l4()

    for pool in (psum_bm, psum_mm, scratch, outs, acts, consts):
        pool.release()


_PST_STATE = {}


def _pst(nc, scratch, outs, bm, half, j):
    """PST epilogue on half of a batch-major [128, 8, 64] PSUM tile.

    `bm` holds v4+b4 batch-major. out = t + relu(t)*(1/denom - 1), which
    equals where(t>0, t/denom, t): for t<=0 relu(t)=0; when possum==0 the
    denom fix makes the correction factor 0. Returns the output SBUF tile
    (complete when half == 1).
    """
    G = BLOCKS_PER_BM // 2
    Tanh = mybir.ActivationFunctionType.Tanh
    Sigm = mybir.ActivationFunctionType.Sigmoid

    if half == 0:
        _PST_STATE[j] = outs.tile([128, 2 * G, D_OUT], F32, tag="o", name="o_sb")
    o_full = _PST_STATE.pop(j) if half == 1 else _PST_STATE[j]
    o_sb = o_full[:, half * G:(half + 1) * G, :]
    bmh = bm[:, half * G:(half + 1) * G, :]

    nc.scalar.activation(out=o_sb[:, :, 0:63], in_=bmh[:, :, 0:63], func=Tanh)
    nc.scalar.activation(out=o_sb[:, :, 63:64], in_=bmh[:, :, 63:64], func=Sigm)

    tv = o_sb[:, :, 0:63]  # tanh part [128, G, 63]
    # rl/corr must stay f32: out = t + rl*rm1 cancels to ~t/15, so any
    # rounding in rl or corr is amplified ~30x on the positive outputs.
    rl = scratch.tile([128, G, 63], F32, tag="rl")
    nc.vector.tensor_scalar_max(rl, tv, 0.0)
    possum = scratch.tile([128, G], F32, tag="possum")
    nc.vector.reduce_sum(out=possum, in_=rl, axis=mybir.AxisListType.X)
    denom = scratch.tile([128, G], F32, tag="denom")
    nc.vector.scalar_tensor_tensor(out=denom, in0=possum, scalar=0.0,
                                   in1=possum, op0=mybir.AluOpType.is_equal,
                                   op1=mybir.AluOpType.add)
    recip = scratch.tile([128, G], F32, tag="recip")
    nc.vector.reciprocal(recip, denom)
    rm1 = scratch.tile([128, G], F32, tag="rm1")
    nc.vector.tensor_scalar(out=rm1, in0=recip, scalar1=-1.0, scalar2=None,
                            op0=mybir.AluOpType.add)
    corr = scratch.tile([128, G, 63], F32, tag="corr")
    nc.vector.tensor_tensor(
        out=corr, in0=rl, in1=rm1.unsqueeze(2).broadcast_to([128, G, 63]),
        op=mybir.AluOpType.mult)
    nc.vector.tensor_tensor(out=o_sb[:, :, 0:63], in0=tv, in1=corr,
                            op=mybir.AluOpType.add)
    return o_full


_PROG_CACHE = {}


def _build():
    if "nc" in _PROG_CACHE:
        return _PROG_CACHE["nc"]
    nc = bacc.Bacc("TRN2", target_bir_lowering=False, debug=False,
                   enable_asserts=False)
    n_j = B // (128 * BLOCKS_PER_BM)  # 8 output tiles of 1024 rows
    aps = {
        "xT": nc.dram_tensor("xT", [D_IN, B], BF16, kind="ExternalInput").ap(),
        "W1": nc.dram_tensor("W1", [D_IN, 512], BF16, kind="ExternalInput").ap(),
        "b1": nc.dram_tensor("b1", [512], F32, kind="ExternalInput").ap(),
        "W2": nc.dram_tensor("W2", [512, 384], BF16, kind="ExternalInput").ap(),
        "W3": nc.dram_tensor("W3", [384, 128], BF16, kind="ExternalInput").ap(),
        "W4": nc.dram_tensor("W4", [128, D_OUT], BF16, kind="ExternalInput").ap(),
        # partition-major: out[p, j, t, f] = row 1024*j + 128*t + p
        "out": nc.dram_tensor("out", [128, n_j, BLOCKS_PER_BM, D_OUT], F32,
                              kind="ExternalOutput").ap(),
    }
    with tile.TileContext(nc) as tc:
        _emit(tc, aps)
    nc.compile()
    _PROG_CACHE["nc"] = nc
    return nc


def kernel(state, W1, b1, W2, b2, W3, b3, W4, b4, _trace=False):
    nc = _build()
    state = np.asarray(state, dtype=np.float32)

    bf16 = ml_dtypes.bfloat16
    f32 = np.float32

    W1p = np.zeros((512, 512), f32)
    W1p[:, :H1] = np.asarray(W1, f32)
    b1p = np.zeros((512,), f32)
    b1p[:H1] = np.asarray(b1, f32)
    b1p[H1] = 1.0  # plants the ones-row in h1
    W2p = np.zeros((512, 384), f32)
    W2p[:H1, :H2] = np.asarray(W2, f32)
    W2p[H1, :H2] = np.asarray(b2, f32)
    W2p[H1, H2] = 1.0  # regenerates the ones-row in h2
    W3p = np.zeros((384, 128), f32)
    W3p[:H2, :H3] = np.asarray(W3, f32)
    W3p[H2, :H3] = np.asarray(b3, f32)
    W3p[H2, H3] = 1.0  # regenerates the ones-row in h3
    W4p = np.zeros((128, D_OUT), f32)
    W4p[:H3] = np.asarray(W4, f32)
    W4p[H3] = np.asarray(b4, f32)

    weights = {
        "W1": W1p.astype(bf16), "b1": b1p,
        "W2": W2p.astype(bf16), "W3": W3p.astype(bf16), "W4": W4p.astype(bf16),
    }
    in_maps = []
    for i in range(N_CORES):
        shard = state[i * B:(i + 1) * B]
        in_maps.append({"xT": np.ascontiguousarray(shard.T).astype(bf16),
                        **weights})

    res = run_bass_kernel_spmd(nc, in_maps, core_ids=list(range(N_CORES)),
                               trace=_trace)
    # invert the partition-major output layout: [128, j, t, f] -> row
    # 1024*j + 128*t + p
    full = np.concatenate(
        [np.transpose(res.results[i]["out"], (1, 2, 0, 3)).reshape(B, D_OUT)
         for i in range(N_CORES)], axis=0)
    if _trace:
        kernel.last_results = res
    return full


# revision 35
# speedup vs baseline: 1.1928x; 1.0156x over previous
"""Trainium2 Bass kernel for the DActor dense MLP.

Network (per row of `state`):
    h1 = relu(state @ W1 + b1)        # 512 -> 500
    h2 = relu(h1 @ W2 + b2)           # 500 -> 300
    h3 = relu(h2 @ W3 + b3)           # 300 -> 100
    v  = h3 @ W4 + b4                 # 100 -> 64
    t  = tanh(v[:, :63]); s = sigmoid(v[:, 63:])
    possum = sum(relu(t)); denom = possum == 0 ? 1 : possum
    out = concat(where(t > 0, t / denom, t), s)

Strategy: pure data parallel over 8 NeuronCores (8192 rows each).
Activations are feature-major ([feat, batch]); weights are the stationary
operand, activations the 512-wide moving operand. All matmul operands are
bf16 (fp32 PSUM accumulation) — same 1 col/cycle PE rate as fp32r but half
the DMA/LDWEIGHTS traffic.

Biases ride inside the matmuls via a ones-row that propagates through the
net: b1 is applied by the ACT engine whose padded bias vector also plants
h1[500] = relu(0 + 1) = 1; W2/W3 are padded with a bias row (row 500/300)
plus a 1.0 diagonal element that regenerates the ones-row in h2/h3. L2/L3
activations are then pure relu and run on the DVE.

L4 is fused with the batch-transpose: per 128-row block, the matmul uses
the feature-major h3 block as the *stationary* operand and W4 (with b4 as
its ones-row) as the 64-column moving operand, writing v4 batch-major
straight into PSUM — no identity transpose, no separate bias pass. The PST
epilogue computes out = t + relu(t) * (1/denom - 1), which equals
where(t>0, t/denom, t) without predicated copies.

The output DRAM tensor is partition-major [128, 8, 8, 64] so each store
is 2KB-contiguous per partition; the host inverts the permutation.

Scheduling notes (measured on trn2 via axon):
- A matmul whose output spans a full 512-element PSUM bank throttles the
  PE to ~2.0GHz; 256-col outputs sustain ~2.4GHz, so every (k, m) pass is
  emitted as two 256-col half-matmuls (accumulation order per output
  element is unchanged, results bit-identical).
- Each dma_start owns one DMA ring (~24GB/s, ~42ns/descriptor) and costs
  ~610ns of descriptor generation on the issuing sequencer; transfers are
  split per k-tile across rings, and wide-but-shallow transfers (b1) are
  split to avoid a single 20us ring queue.
- ~110 junk matmuls warm the PE's DVFS (~3us to full clock) while the
  first chunk's DMAs land.
"""

import ml_dtypes
import numpy as np

import concourse.tile as tile
from concourse import bacc, mybir
from concourse.bass_utils import run_bass_kernel_spmd

N_CORES = 8
BATCH = 65536
B = BATCH // N_CORES  # 8192 rows per core
D_IN, H1, H2, H3, D_OUT = 512, 500, 300, 100, 64
NCHUNK = 512  # moving-operand width (= 1 PSUM bank of fp32)
N_CHUNKS = B // NCHUNK  # 16
BLOCKS_PER_BM = 8  # 128-row blocks per batch-major output tile
N_BM = B // (128 * BLOCKS_PER_BM)  # 8 output tiles of 1024 rows

F32 = mybir.dt.float32
BF16 = mybir.dt.bfloat16

K1, K2, K3 = 4, 4, 3  # k-tiles per layer (501->4x128, 301->3x128 incl bias row)
M1, M2 = 4, 3  # m-tiles for L1 (512 cols) / L2 (384 cols)


def _emit(tc: tile.TileContext, aps: dict):
    nc = tc.nc
    xT = aps["xT"]
    out = aps["out"]  # [128, N_BM, BLOCKS_PER_BM, D_OUT] partition-major

    consts = tc.alloc_tile_pool(name="consts", bufs=1)
    acts = tc.alloc_tile_pool(name="acts", bufs=3)
    outs = tc.alloc_tile_pool(name="outs", bufs=3)
    scratch = tc.alloc_tile_pool(name="scratch", bufs=2)
    psum_mm = tc.alloc_tile_pool(name="psum_mm", bufs=7, space="PSUM")
    psum_bm = tc.alloc_tile_pool(name="psum_bm", bufs=1, space="PSUM")

    # ---- persistent constants -------------------------------------------
    # Weights arrive host-padded (see kernel()): W1 [512,512], W2 [512,384]
    # with row 500 = [b2, 1.0@300], W3 [384,128] with row 300 = [b3, 1.0@100],
    # W4 [128,64] with row 100 = b4. Padded rows/cols are zero so every
    # matmul runs full-K with bit-identical results.
    xT_v = xT.rearrange("(k p) b -> p k b", p=128)  # [128, 4, B]

    # Each dma_start owns one DMA ring (~24GB/s) and costs ~750ns of
    # descriptor-generation on the issuing sequencer, so the first-chunk
    # data (x0 + w1) is split per k-tile across 8 rings, k0 first. w2..w4/b1
    # ride the gpsimd queue and are issued after chunk 0's layer-1 emission
    # (not needed until ~+8us).
    x0_sb = acts.tile([128, K1, NCHUNK], BF16, tag="x", name="x0_sb")
    w1 = consts.tile([128, K1, 512], BF16)
    w1_v = aps["W1"].rearrange("(k p) m -> p k m", p=128)
    for ki in range(K1):
        nc.sync.dma_start(out=x0_sb[:, ki, :], in_=xT_v[:, ki, 0:NCHUNK])
        nc.sync.dma_start(out=w1[:, ki, :], in_=w1_v[:, ki, :])
    w2 = consts.tile([128, K2, 384], BF16)
    w3 = consts.tile([128, K3, 128], BF16)
    w4 = consts.tile([128, D_OUT], BF16)
    # b1 padded with b1[500] = 1.0: relu(psum_pad + 1) plants the ones-row
    # in h1 that carries the biases through L2/L3/L4.
    b1 = consts.tile([128, M1], F32)

    def load_late_consts():
        # split across rings: a single dma_start owns one ~24GB/s ring and
        # processes descriptors at ~42ns each, so the 512-descriptor w2/b1
        # transfers would otherwise land ~20us late
        b1_v = aps["b1"].rearrange("(m p) -> p m", p=128)
        for mi in range(M1):
            nc.gpsimd.dma_start(out=b1[:, mi:mi + 1], in_=b1_v[:, mi:mi + 1])
        w2_v = aps["W2"].rearrange("(k p) m -> p k m", p=128)
        for ki in range(K2):
            nc.gpsimd.dma_start(out=w2[:, ki, :], in_=w2_v[:, ki, :])
        w3_v = aps["W3"].rearrange("(k p) m -> p k m", p=128)
        for ki in range(K3):
            nc.gpsimd.dma_start(out=w3[:, ki, :], in_=w3_v[:, ki, :])
        nc.gpsimd.dma_start(out=w4, in_=aps["W4"])

    Relu = mybir.ActivationFunctionType.Relu

    # Warm the PE while the first DMAs land: the tensor engine's DVFS needs
    # ~3us of continuous execution to reach full clock, and these junk
    # matmuls (zero stationary/moving, result never read) have no input
    # dependencies, so the PE ramps during the DMA head instead of on the
    # first real chunks.
    wu = consts.tile([128, 64], BF16)
    nc.gpsimd.memset(wu, 0.0)
    wu_ps = psum_bm.tile([128, 8, D_OUT], F32, tag="bm", name="wu_ps")
    for i in range(110):
        nc.tensor.matmul(wu_ps[:64, i % 8, :], wu, wu, start=True, stop=True)

    def mm(ps, lhsT, rhs, start, stop):
        nc.tensor.matmul(ps, lhsT, rhs, start=start, stop=stop)

    pending_l4 = None  # (chunk, h3 tile) whose L4 matmuls are deferred
    bm_state = [None]  # current batch-major psum tile

    def emit_l4():
        # L4 for the previous chunk, emitted after the next chunk's L1
        # matmuls so the PE never waits on the DVE-produced h3. Stationary
        # operand is the feature-major h3 block; moving operand is W4
        # (64 cols), writing v4+b4 batch-major straight into PSUM.
        nonlocal pending_l4
        if pending_l4 is None:
            return
        pc, ph3 = pending_l4
        pending_l4 = None
        for bb in range(NCHUNK // 128):
            g = pc * (NCHUNK // 128) + bb  # global 128-row block index
            t = g % BLOCKS_PER_BM
            if t == 0:
                bm_state[0] = psum_bm.tile([128, BLOCKS_PER_BM, D_OUT], F32,
                                           tag="bm", name="bm")
            bm = bm_state[0]
            mm(bm[:, t, :], ph3[:, bb * 128:(bb + 1) * 128], w4,
               start=True, stop=True)
            # PST runs in two half-groups: the first half fires a chunk
            # earlier (hidden under compute), halving the serial tail after
            # the final matmul.
            if t == BLOCKS_PER_BM // 2 - 1:
                _pst(nc, scratch, outs, bm, 0, g // BLOCKS_PER_BM)
            elif t == BLOCKS_PER_BM - 1:
                j = g // BLOCKS_PER_BM
                o_sb = _pst(nc, scratch, outs, bm, 1, j)
                if j == B // (128 * BLOCKS_PER_BM) - 1:
                    # the final store is tail-critical: split it across 4
                    # rings (a single ring moves only ~24GB/s), with the
                    # ~750ns trigger generation split across two sequencers
                    for q in range(4):
                        eng = nc.sync if q % 2 == 0 else nc.scalar
                        eng.dma_start(out=out[32 * q:32 * (q + 1), j],
                                      in_=o_sb[32 * q:32 * (q + 1)])
                else:
                    nc.sync.dma_start(out=out[:, j], in_=o_sb)

    for c in range(N_CHUNKS):
        cs = slice(c * NCHUNK, (c + 1) * NCHUNK)

        if c == 0:
            x_sb = x0_sb
        else:
            # Rings process FIFO, so these queue behind the chunk-0 data
            # without delaying it.
            x_sb = acts.tile([128, K1, NCHUNK], BF16, tag="x")
            for ki in range(K1):
                nc.sync.dma_start(out=x_sb[:, ki, :], in_=xT_v[:, ki, cs])

        # two 256-col half-matmuls per (k, m) pass — see module docstring
        HALF = NCHUNK // 2

        # ---- layer 1: [512 -> 500(+ones row, pad 512)] ------------------
        h1 = acts.tile([128, K2, NCHUNK], BF16, tag="h1")
        for mi in range(M1):
            ps = psum_mm.tile([128, NCHUNK], F32, tag="ps")
            msl = slice(mi * 128, (mi + 1) * 128)
            for ki in range(K1):
                for h in range(2):
                    hs = slice(h * HALF, (h + 1) * HALF)
                    mm(ps[:, hs], w1[:, ki, msl], x_sb[:, ki, hs],
                       start=(ki == 0), stop=(ki == K1 - 1))
            nc.scalar.activation(out=h1[:, mi, :], in_=ps, func=Relu,
                                 bias=b1[:, mi:mi + 1])
        if c == 0:
            load_late_consts()
        emit_l4()

        # ---- layer 2: [501 -> 300(+ones row, pad 384)] ------------------
        h2 = acts.tile([128, K3, NCHUNK], BF16, tag="h2")
        for mi in range(M2):
            ps = psum_mm.tile([128, NCHUNK], F32, tag="ps")
            msl = slice(mi * 128, (mi + 1) * 128)
            for ki in range(K2):
                for h in range(2):
                    hs = slice(h * HALF, (h + 1) * HALF)
                    mm(ps[:, hs], w2[:, ki, msl], h1[:, ki, hs],
                       start=(ki == 0), stop=(ki == K2 - 1))
            nc.vector.tensor_scalar_max(h2[:, mi, :], ps, 0.0)

        # ---- layer 3: [301 -> 100(+ones row, pad 128)] ------------------
        h3 = acts.tile([128, NCHUNK], BF16, tag="h3")
        ps = psum_mm.tile([128, NCHUNK], F32, tag="ps")
        for ki in range(K3):
            for h in range(2):
                hs = slice(h * HALF, (h + 1) * HALF)
                mm(ps[:, hs], w3[:, ki, :], h2[:, ki, hs],
                   start=(ki == 0), stop=(ki == K3 - 1))
        nc.vector.tensor_scalar_max(h3, ps, 0.0)

        pending_l4 = (c, h3)

    emit_l4()

    for pool in (psum_bm, psum_mm, scratch, outs, acts, consts):
        pool.release()


_PST_STATE = {}


def _pst(nc, scratch, outs, bm, half, j):
    """PST epilogue on half of a batch-major [128, 8, 64] PSUM tile.

    `bm` holds v4+b4 batch-major. out = t + relu(t)*(1/denom - 1), which
    equals where(t>0, t/denom, t): for t<=0 relu(t)=0; when possum==0 the
    denom fix makes the correction factor 0. Returns the output SBUF tile
    (complete when half == 1).
    """
    G = BLOCKS_PER_BM // 2
    Tanh = mybir.ActivationFunctionType.Tanh
    Sigm = mybir.ActivationFunctionType.Sigmoid

    if half == 0:
        _PST_STATE[j] = outs.tile([128, 2 * G, D_OUT], F32, tag="o", name="o_sb")
    o_full = _PST_STATE.pop(j) if half == 1 else _PST_STATE[j]
    o_sb = o_full[:, half * G:(half + 1) * G, :]
    bmh = bm[:, half * G:(half + 1) * G, :]

    nc.scalar.activation(out=o_sb[:, :, 0:63], in_=bmh[:, :, 0:63], func=Tanh)
    nc.scalar.activation(out=o_sb[:, :, 63:64], in_=bmh[:, :, 63:64], func=Sigm)

    tv = o_sb[:, :, 0:63]  # tanh part [128, G, 63]
    # rl/corr must stay f32: out = t + rl*rm1 cancels to ~t/15, so any
    # rounding in rl or corr is amplified ~30x on the positive outputs.
    rl = scratch.tile([128, G, 63], F32, tag="rl")
    nc.vector.tensor_scalar_max(rl, tv, 0.0)
    possum = scratch.tile([128, G], F32, tag="possum")
    nc.vector.reduce_sum(out=possum, in_=rl, axis=mybir.AxisListType.X)
    denom = scratch.tile([128, G], F32, tag="denom")
    nc.vector.scalar_tensor_tensor(out=denom, in0=possum, scalar=0.0,
                                   in1=possum, op0=mybir.AluOpType.is_equal,
                                   op1=mybir.AluOpType.add)
    recip = scratch.tile([128, G], F32, tag="recip")
    nc.vector.reciprocal(recip, denom)
    rm1 = scratch.tile([128, G], F32, tag="rm1")
    nc.vector.tensor_scalar(out=rm1, in0=recip, scalar1=-1.0, scalar2=None,
                            op0=mybir.AluOpType.add)
    corr = scratch.tile([128, G, 63], F32, tag="corr")
    nc.vector.tensor_tensor(
        out=corr, in0=rl, in1=rm1.unsqueeze(2).broadcast_to([128, G, 63]),
        op=mybir.AluOpType.mult)
    nc.vector.tensor_tensor(out=o_sb[:, :, 0:63], in0=tv, in1=corr,
                            op=mybir.AluOpType.add)
    return o_full


_PROG_CACHE = {}


def _build():
    if "nc" in _PROG_CACHE:
        return _PROG_CACHE["nc"]
    nc = bacc.Bacc("TRN2", target_bir_lowering=False, debug=False,
                   enable_asserts=False)
    n_j = B // (128 * BLOCKS_PER_BM)  # 8 output tiles of 1024 rows
    aps = {
        "xT": nc.dram_tensor("xT", [D_IN, B], BF16, kind="ExternalInput").ap(),
        "W1": nc.dram_tensor("W1", [D_IN, 512], BF16, kind="ExternalInput").ap(),
        "b1": nc.dram_tensor("b1", [512], F32, kind="ExternalInput").ap(),
        "W2": nc.dram_tensor("W2", [512, 384], BF16, kind="ExternalInput").ap(),
        "W3": nc.dram_tensor("W3", [384, 128], BF16, kind="ExternalInput").ap(),
        "W4": nc.dram_tensor("W4", [128, D_OUT], BF16, kind="ExternalInput").ap(),
        # partition-major: out[p, j, t, f] = row 1024*j + 128*t + p
        "out": nc.dram_tensor("out", [128, n_j, BLOCKS_PER_BM, D_OUT], F32,
                              kind="ExternalOutput").ap(),
    }
    with tile.TileContext(nc) as tc:
        _emit(tc, aps)
    nc.compile()
    _PROG_CACHE["nc"] = nc
    return nc


def kernel(state, W1, b1, W2, b2, W3, b3, W4, b4, _trace=False):
    nc = _build()
    state = np.asarray(state, dtype=np.float32)

    bf16 = ml_dtypes.bfloat16
    f32 = np.float32

    W1p = np.zeros((512, 512), f32)
    W1p[:, :H1] = np.asarray(W1, f32)
    b1p = np.zeros((512,), f32)
    b1p[:H1] = np.asarray(b1, f32)
    b1p[H1] = 1.0  # plants the ones-row in h1
    W2p = np.zeros((512, 384), f32)
    W2p[:H1, :H2] = np.asarray(W2, f32)
    W2p[H1, :H2] = np.asarray(b2, f32)
    W2p[H1, H2] = 1.0  # regenerates the ones-row in h2
    W3p = np.zeros((384, 128), f32)
    W3p[:H2, :H3] = np.asarray(W3, f32)
    W3p[H2, :H3] = np.asarray(b3, f32)
    W3p[H2, H3] = 1.0  # regenerates the ones-row in h3
    W4p = np.zeros((128, D_OUT), f32)
    W4p[:H3] = np.asarray(W4, f32)
    W4p[H3] = np.asarray(b4, f32)

    weights = {
        "W1": W1p.astype(bf16), "b1": b1p,
        "W2": W2p.astype(bf16), "W3": W3p.astype(bf16), "W4": W4p.astype(bf16),
    }
    in_maps = []
    for i in range(N_CORES):
        shard = state[i * B:(i + 1) * B]
        in_maps.append({"xT": np.ascontiguousarray(shard.T).astype(bf16),
                        **weights})

    res = run_bass_kernel_spmd(nc, in_maps, core_ids=list(range(N_CORES)),
                               trace=_trace)
    # invert the partition-major output layout: [128, j, t, f] -> row
    # 1024*j + 128*t + p
    full = np.concatenate(
        [np.transpose(res.results[i]["out"], (1, 2, 0, 3)).reshape(B, D_OUT)
         for i in range(N_CORES)], axis=0)
    if _trace:
        kernel.last_results = res
    return full
